# revision 1
# baseline (speedup 1.0000x reference)
"""Bahdanau-attention scoring kernel for one TRN2 chip (8 NeuronCores).

Computes softmax_L(v . tanh(enc @ W1^T + hidden @ W2^T + b1 + b2)) for
B=16, L=4096, H=1024, data-parallel over B (2 batches per core, no
collectives; softmax axis L stays core-local).

Host-side prep (cheap layout work, <0.03% of FLOPs):
  - W1 is pre-transposed to [h, o] so it DMAs straight into the h-partitioned
    layout the TensorEngine contraction needs.
  - The per-(batch, o) additive bias  w1_b + w2_b + hidden @ W2^T  is folded
    into one small [128, 8, 2] tensor consumed as the ScalarE activation bias.
  - mask is pre-scaled to -1e10 * mask so it folds into the energy PSUM via a
    rank-1 matmul.

Device per core (B_loc=2, 8192 x 1024 encoder shard):
  per 512-row stripe: DMA in -> 32 PE transposes (enc^T) -> drain to SBUF ->
  8x8 matmuls (enc @ W1^T, float32r fast-fp32 path) -> fused bias+tanh on
  ScalarE -> v-dot + mask as 9 small matmuls into a [1,512] PSUM -> Exp with
  accumulated row-sum. Tail: reciprocal + scale + DMA out.
"""

import os
import sys

import numpy as np

_REPO = "/opt/trn_rl_repo"
if _REPO not in sys.path:
    sys.path.insert(0, _REPO)

B, L, H = 16, 4096, 1024
NCORES = 8
B_LOC = B // NCORES  # 2
ROWS = B_LOC * L  # 8192
NEG = -1.0e10
P = 128
LSUP = 512  # l-positions per psum tile
NSUP = ROWS // LSUP  # 16 stripes
SUBS = LSUP // P  # 4 transpose blocks per stripe
KC = H // P  # 8 contraction chunks
OC = H // P  # 8 output chunks

COMPUTE = os.environ.get("ATTN_COMPUTE", "bf16")  # bf16 | f32r | f32
TRANSPOSE = os.environ.get("ATTN_TRANSPOSE", "dma")  # dma | pe  (dma needs bf16)
LAYOUT = os.environ.get("ATTN_LAYOUT", "b")  # b: enc_e=[o,l], v-dot on PE (verified)
#                                             a: enc_e=[l,o], v-dot on DVE (fails on HW)
VERSION = float(os.environ.get("ATTN_VER", "15"))  # cache-buster / run marker
REPEAT = int(os.environ.get("ATTN_REPEAT", "1"))  # body replicas (timing only)
DEBUG_LVL = int(os.environ.get("ATTN_DEBUG", "0"))  # 0=off 1=no tail 2=+no epi 3=+no ttr


def _np_io_dtype():
    if COMPUTE == "bf16":
        import ml_dtypes

        return np.dtype(ml_dtypes.bfloat16)
    return np.dtype(np.float32)


def _build():
    if LAYOUT == "a":
        return _build_a()
    return _build_b()


def _build_a():
    """enc_e in [l, o] layout: stationary = transposed-encoder blocks,
    moving = W1^T. The v-dot runs on VectorE (tensor_tensor_reduce), the
    bias is a broadcast add on VectorE, so the TensorEngine runs ONLY the
    64 main matmuls per stripe (plus one tiny [128,4] energy transpose)."""
    from contextlib import ExitStack

    import concourse.bass as bass
    import concourse.bass_isa as bass_isa
    import concourse.mybir as mybir
    import concourse.tile as tile
    from concourse import bacc
    from concourse.bass import ts
    from concourse.masks import make_identity

    F32 = mybir.dt.float32
    BF16 = mybir.dt.bfloat16
    assert COMPUTE == "bf16" and TRANSPOSE == "dma", "layout a requires bf16+dma"

    nc = bacc.Bacc("TRN2", target_bir_lowering=False, debug=False)
    enc_d = nc.dram_tensor("enc", [ROWS, H], BF16, kind="ExternalInput").ap()
    w1t_d = nc.dram_tensor("w1t", [H, H], BF16, kind="ExternalInput").ap()
    cbias_d = nc.dram_tensor("cbias", [B_LOC, H], F32, kind="ExternalInput").ap()
    v_d = nc.dram_tensor("vt", [1, H], BF16, kind="ExternalInput").ap()
    SPB = NSUP // B_LOC  # stripes per batch
    # mask and out live in the device-native [lt, stripe, p] layout; the host
    # permutes to/from [b, l] (l = (s % SPB)*512 + lt*128 + p).
    maskneg_d = nc.dram_tensor(
        "maskneg", [SUBS, NSUP, P], F32, kind="ExternalInput"
    ).ap()
    out_d = nc.dram_tensor("out", [SUBS, NSUP, P], F32, kind="ExternalOutput").ap()
    ver_d = nc.dram_tensor("ver", [1, 1], F32, kind="ExternalOutput").ap()

    Tanh = mybir.ActivationFunctionType.Tanh
    Exp = mybir.ActivationFunctionType.Exp
    HALF = 512

    with tile.TileContext(nc) as tc, ExitStack() as ctx:
        consts = ctx.enter_context(tc.tile_pool(name="consts", bufs=1))
        w1t_pool = ctx.enter_context(tc.tile_pool(name="w1t", bufs=1))
        enct_pool = ctx.enter_context(tc.tile_pool(name="enct", bufs=32))
        tmp_pool = ctx.enter_context(tc.tile_pool(name="tmp", bufs=4))
        tanh_pool = ctx.enter_context(tc.tile_pool(name="tanh", bufs=3))
        scr_pool = ctx.enter_context(tc.tile_pool(name="scr", bufs=2))
        se_pool = ctx.enter_context(tc.tile_pool(name="se", bufs=3))
        ps_mm = ctx.enter_context(tc.tile_pool(name="ps_mm", bufs=5, space="PSUM"))
        ps_tr = ctx.enter_context(tc.tile_pool(name="ps_tr", bufs=2, space="PSUM"))
        ps_sm = ctx.enter_context(tc.tile_pool(name="ps_sm", bufs=1, space="PSUM"))

        # ---- constants / small inputs ----
        identf = consts.tile([P, P], F32)
        make_identity(nc, identf[:, :])
        ones_row_b = consts.tile([1, P], BF16)
        nc.vector.memset(ones_row_b[:, :], 1.0)
        ones_row_f = consts.tile([1, P], F32)
        nc.vector.memset(ones_row_f[:, :], 1.0)
        ones_col_f = consts.tile([P, 1], F32)
        nc.vector.memset(ones_col_f[:, :], 1.0)
        ver_sb = consts.tile([1, 1], F32)
        nc.vector.memset(ver_sb[:, :], VERSION)
        nc.sync.dma_start(out=ver_d[:, :], in_=ver_sb[:, :])

        w1t_sb = []
        for hc in range(KC):
            t = w1t_pool.tile([P, H], BF16, tag=f"w1t{hc}")
            nc.sync.dma_start(out=t[:, :], in_=w1t_d[ts(hc, P), :])
            w1t_sb.append(t)

        cbias_row = consts.tile([1, B_LOC, H], F32)
        nc.sync.dma_start(
            out=cbias_row[:, :, :], in_=cbias_d[:, :].rearrange("b h -> () b h")
        )
        v_row = consts.tile([1, H], BF16)
        nc.sync.dma_start(out=v_row[:, :], in_=v_d[:, :])
        mask_t = consts.tile([SUBS, NSUP, P], F32)
        nc.sync.dma_start(out=mask_t[:, :, :], in_=maskneg_d[:, :, :])

        # materialize partition-broadcast tiles via rank-1 matmuls (one-time)
        v_bcast = consts.tile([P, H], BF16)
        cb_bcast = [
            consts.tile([P, H], F32, tag=f"cbb{b}", name=f"cbb{b}")
            for b in range(B_LOC)
        ]
        if DEBUG_LVL >= 4:
            nc.vector.memset(v_bcast[:, :], 0.03)
            for b in range(B_LOC):
                nc.vector.memset(cb_bcast[b][:, :], 0.1)
        else:
            # partition-broadcast via DMA: every partition reads the same
            # DRAM row (stride-0 partition dim on the source AP)
            nc.sync.dma_start(
                out=v_bcast[:, :], in_=v_d[:, :].broadcast_to([P, H])
            )
            for b in range(B_LOC):
                nc.sync.dma_start(
                    out=cb_bcast[b][:, :],
                    in_=cbias_d[b : b + 1, :].broadcast_to([P, H]),
                )

        punorm_t = consts.tile([SUBS, NSUP, P], F32)
        sums_t = consts.tile([SUBS, NSUP], F32)
        if DEBUG_LVL >= 2:
            nc.vector.memset(punorm_t[:, :, :], 0.5)

        for _rep in range(REPEAT):
            # ---- main loop over 512-row stripes ----
            for s in range(NSUP):
                b = s // SPB

                enct = []
                for hc in range(KC):
                    et = enct_pool.tile([P, LSUP], BF16, tag="et")
                    nc.sync.dma_start(
                        out=et[:, :],
                        in_=enc_d[bass.ds(s * LSUP, LSUP), ts(hc, P)],
                        transpose=True,
                    )
                    enct.append(et)

                stripe_e = se_pool.tile([P, SUBS], F32, tag="se")
                for lt in range(SUBS):
                    th = tanh_pool.tile([P, H], BF16, tag="th")
                    for half in range(2):
                        pmm = ps_mm.tile([P, HALF], F32, tag="pmm")
                        for hc in range(KC):
                            nc.tensor.matmul(
                                out=pmm[:, :],
                                lhsT=enct[hc][:, ts(lt, P)],
                                rhs=w1t_sb[hc][:, ts(half, HALF)],
                                start=(hc == 0),
                                stop=(hc == KC - 1),
                            )
                        tmp = tmp_pool.tile([P, HALF], F32, tag="tmp")
                        nc.vector.tensor_add(
                            tmp[:, :], pmm[:, :], cb_bcast[b][:, ts(half, HALF)]
                        )
                        nc.scalar.activation(th[:, ts(half, HALF)], tmp[:, :], Tanh)
                    scr = scr_pool.tile([P, H], F32, tag="scr")
                    if DEBUG_LVL >= 3:
                        nc.vector.tensor_copy(scr[:, ts(0, HALF)], th[:, ts(0, HALF)])
                    else:
                        nc.vector.tensor_tensor_reduce(
                            out=scr[:, :],
                            in0=th[:, :],
                            in1=v_bcast[:, :],
                            scale=1.0,
                            scalar=0.0,
                            op0=mybir.AluOpType.mult,
                            op1=mybir.AluOpType.add,
                            accum_out=stripe_e[:, lt : lt + 1],
                        )

                if DEBUG_LVL >= 2:
                    continue
                # relayout the 512 energies to partitions=lt via PE transpose
                pse = ps_tr.tile([SUBS, P], F32, tag="pse")
                nc.tensor.transpose(pse[:, :], stripe_e[:, :], identf[:, :])
                em = tmp_pool.tile([SUBS, P], F32, tag="em")
                nc.vector.tensor_add(em[:, :], pse[:, :], mask_t[:, s, :])
                nc.scalar.activation(
                    punorm_t[:, s, :],
                    em[:, :],
                    Exp,
                    accum_out=sums_t[:, s : s + 1],
                )

            # ---- normalize in place and store ----
            if DEBUG_LVL >= 1:
                nc.sync.dma_start(out=out_d[:, :, :], in_=punorm_t[:, :, :])
                continue
            for b in range(B_LOC):
                tot4 = consts.tile([SUBS, 1], F32, tag="tot4")
                nc.vector.tensor_reduce(
                    out=tot4[:, :],
                    in_=sums_t[:, ts(b, SPB)],
                    axis=mybir.AxisListType.X,
                    op=mybir.AluOpType.add,
                )
                zall = consts.tile([SUBS, 1], F32, tag="zall")
                nc.gpsimd.partition_all_reduce(
                    zall[:, :], tot4[:, :], channels=SUBS,
                    reduce_op=bass_isa.ReduceOp.add,
                )
                rzb = consts.tile([SUBS, 1], F32, tag="rzb")
                nc.vector.reciprocal(rzb[:, :], zall[:, :])
                nc.vector.tensor_scalar_mul(
                    punorm_t[:, ts(b, SPB), :],
                    punorm_t[:, ts(b, SPB), :],
                    rzb[:, :],
                )
            nc.sync.dma_start(out=out_d[:, :, :], in_=punorm_t[:, :, :])

    nc.compile()
    return nc


def _build_b():
    from contextlib import ExitStack

    import concourse.bass as bass
    import concourse.mybir as mybir
    import concourse.tile as tile
    from concourse import bacc
    from concourse.bass import ts
    from concourse.masks import make_identity

    F32 = mybir.dt.float32
    F32R = mybir.dt.float32r
    BF16 = mybir.dt.bfloat16

    io_dt = BF16 if COMPUTE == "bf16" else F32

    def mm(ap):
        # engine-facing dtype for matmul operands
        return ap.bitcast(F32R) if COMPUTE == "f32r" else ap

    nc = bacc.Bacc("TRN2", target_bir_lowering=False, debug=False)
    enc_d = nc.dram_tensor("enc", [ROWS, H], io_dt, kind="ExternalInput").ap()
    w1t_d = nc.dram_tensor("w1t", [H, H], io_dt, kind="ExternalInput").ap()
    cbias_d = nc.dram_tensor("cbias", [P, OC, B_LOC], F32, kind="ExternalInput").ap()
    vt_d = nc.dram_tensor("vt", [P, OC], io_dt, kind="ExternalInput").ap()
    maskneg_d = nc.dram_tensor(
        "maskneg", [B_LOC, L], io_dt, kind="ExternalInput"
    ).ap()
    out_d = nc.dram_tensor("out", [B_LOC, L], F32, kind="ExternalOutput").ap()
    ver_d = nc.dram_tensor("ver", [1, 1], F32, kind="ExternalOutput").ap()

    Tanh = mybir.ActivationFunctionType.Tanh
    Exp = mybir.ActivationFunctionType.Exp

    with tile.TileContext(nc) as tc, ExitStack() as ctx:
        use_dma_t0 = TRANSPOSE == "dma" and COMPUTE == "bf16"
        consts = ctx.enter_context(tc.tile_pool(name="consts", bufs=1))
        w1t_pool = ctx.enter_context(tc.tile_pool(name="w1t", bufs=1))
        enc_pool = ctx.enter_context(tc.tile_pool(name="enc", bufs=2))
        enct_pool = ctx.enter_context(
            tc.tile_pool(name="enct", bufs=32 if use_dma_t0 else 18)
        )
        tanh_pool = ctx.enter_context(tc.tile_pool(name="tanh", bufs=6))
        ps_tr = ctx.enter_context(
            tc.tile_pool(name="ps_tr", bufs=1 if use_dma_t0 else 3, space="PSUM")
        )
        ps_mm = ctx.enter_context(
            tc.tile_pool(name="ps_mm", bufs=5 if use_dma_t0 else 3, space="PSUM")
        )
        ps_en = ctx.enter_context(tc.tile_pool(name="ps_en", bufs=2, space="PSUM"))

        use_dma_t = TRANSPOSE == "dma" and COMPUTE == "bf16"

        # ---- constants / small inputs ----
        if not use_dma_t:
            ident = consts.tile([P, P], io_dt)
            make_identity(nc, ident[:, :])
        ones = consts.tile([1, 1], io_dt)
        nc.vector.memset(ones[:, :], 1.0)
        ver_sb = consts.tile([1, 1], F32)
        nc.vector.memset(ver_sb[:, :], VERSION)
        nc.sync.dma_start(out=ver_d[:, :], in_=ver_sb[:, :])

        w1t_sb = []
        for hc in range(KC):
            t = w1t_pool.tile([P, H], io_dt, tag=f"w1t{hc}")
            nc.sync.dma_start(out=t[:, :], in_=w1t_d[ts(hc, P), :])
            w1t_sb.append(t)

        cbias_sb = consts.tile([P, OC, B_LOC], F32)
        nc.sync.dma_start(out=cbias_sb[:, :, :], in_=cbias_d[:, :, :])
        vt_sb = consts.tile([P, OC], io_dt)
        nc.sync.dma_start(out=vt_sb[:, :], in_=vt_d[:, :])
        maskneg_sb = consts.tile([1, B_LOC, L], io_dt)
        nc.sync.dma_start(
            out=maskneg_sb[:, :, :],
            in_=maskneg_d[:, :].rearrange("b l -> () b l"),
        )

        punorm = consts.tile([1, B_LOC, L], F32)
        sums = consts.tile([1, NSUP], F32)

        # ---- main loop over 512-row stripes ----
        for _rep in range(REPEAT):
            _main_body(
                nc, tc, bass, mybir, mm, io_dt, F32, consts,
                w1t_sb, cbias_sb, vt_sb, maskneg_sb, punorm, sums,
                enc_d, out_d, enc_pool, enct_pool, tanh_pool,
                ps_tr, ps_mm, ps_en, ones,
                ident if not use_dma_t else None, use_dma_t,
            )

    nc.compile()
    return nc


def _main_body(
    nc, tc, bass, mybir, mm, io_dt, F32, consts,
    w1t_sb, cbias_sb, vt_sb, maskneg_sb, punorm, sums,
    enc_d, out_d, enc_pool, enct_pool, tanh_pool,
    ps_tr, ps_mm, ps_en, ones, ident, use_dma_t,
):
    from concourse.bass import ts

    Tanh = mybir.ActivationFunctionType.Tanh
    Exp = mybir.ActivationFunctionType.Exp
    if True:
        for s in range(NSUP):
            b = s // (NSUP // B_LOC)
            sl = s % (NSUP // B_LOC)

            # bring the 512 x 1024 stripe into h-partitioned (transposed) layout
            enct = []
            if use_dma_t:
                for hc in range(KC):
                    et = enct_pool.tile([P, LSUP], io_dt, tag="et")
                    nc.sync.dma_start(
                        out=et[:, :],
                        in_=enc_d[bass.ds(s * LSUP, LSUP), ts(hc, P)],
                        transpose=True,
                    )
                    enct.append(et)
            else:
                enc_t = enc_pool.tile([P, SUBS, H], io_dt, tag="enc")
                nc.sync.dma_start(
                    out=enc_t[:, :, :],
                    in_=enc_d[bass.ds(s * LSUP, LSUP), :].rearrange(
                        "(sub p) h -> p sub h", p=P
                    ),
                )
                for hc in range(KC):
                    pst = ps_tr.tile([P, LSUP], io_dt, tag="pst")
                    for sub in range(SUBS):
                        nc.tensor.transpose(
                            mm(pst[:, ts(sub, P)]),
                            mm(enc_t[:, sub, ts(hc, P)]),
                            mm(ident[:, :]),
                        )
                    et = enct_pool.tile([P, LSUP], io_dt, tag="et")
                    # alternate drain engine so ScalarE and VectorE split the
                    # PSUM->SBUF copies and the matmuls aren't drain-gated
                    if hc % 2 == 0:
                        nc.vector.tensor_copy(et[:, :], pst[:, :])
                    else:
                        nc.scalar.copy(et[:, :], pst[:, :])
                    enct.append(et)

            # enc @ W1^T  (8 output chunks x 8 contraction chunks)
            tanhs = []
            for oc in range(OC):
                pmm = ps_mm.tile([P, LSUP], F32, tag="pmm")
                for hc in range(KC):
                    nc.tensor.matmul(
                        out=pmm[:, :],
                        lhsT=mm(w1t_sb[hc][:, ts(oc, P)]),
                        rhs=mm(enct[hc][:, :]),
                        start=(hc == 0),
                        stop=(hc == KC - 1),
                    )
                th = tanh_pool.tile([P, LSUP], io_dt, tag="th")
                nc.scalar.activation(
                    th[:, :],
                    pmm[:, :],
                    Tanh,
                    bias=cbias_sb[:, oc, b : b + 1],
                )
                tanhs.append(th)

            # energy row: -1e10*mask + sum_o v_o * tanh[o, l]
            pen = ps_en.tile([1, LSUP], F32, tag="pen")
            nc.tensor.matmul(
                out=pen[:, :],
                lhsT=mm(ones[:, :]),
                rhs=mm(maskneg_sb[:, b, ts(sl, LSUP)]),
                start=True,
                stop=False,
            )
            for oc in range(OC):
                nc.tensor.matmul(
                    out=pen[:, :],
                    lhsT=mm(vt_sb[:, oc : oc + 1]),
                    rhs=mm(tanhs[oc][:, :]),
                    start=False,
                    stop=(oc == OC - 1),
                )

            nc.scalar.activation(
                punorm[:, b, ts(sl, LSUP)],
                pen[:, :],
                Exp,
                accum_out=sums[:, s : s + 1],
            )

        # ---- normalize in place and store ----
        tot = consts.tile([1, B_LOC], F32)
        rec = consts.tile([1, B_LOC], F32)
        for b in range(B_LOC):
            nc.vector.tensor_reduce(
                out=tot[:, b : b + 1],
                in_=sums[:, ts(b, NSUP // B_LOC)],
                axis=mybir.AxisListType.X,
                op=mybir.AluOpType.add,
            )
            nc.vector.reciprocal(rec[:, b : b + 1], tot[:, b : b + 1])
            nc.scalar.mul(punorm[:, b, :], punorm[:, b, :], rec[:, b : b + 1])
        nc.sync.dma_start(
            out=out_d[:, :].rearrange("b l -> () b l"), in_=punorm[:, :, :]
        )


import concourse.bass as bass  # noqa: E402  (after sys.path fix)


def _prep_in_maps(encoder_outputs, hidden, mask, w1_w, w1_b, w2_w, w2_b, v_w):
    io_np = _np_io_dtype()
    enc = np.asarray(encoder_outputs, dtype=np.float32)
    hid = np.asarray(hidden, dtype=np.float32)[:, 0, :]  # [B, H]
    msk = np.asarray(mask)
    w1 = np.asarray(w1_w, dtype=np.float32)
    b1 = np.asarray(w1_b, dtype=np.float32)
    w2 = np.asarray(w2_w, dtype=np.float32)
    b2 = np.asarray(w2_b, dtype=np.float32)
    v = np.asarray(v_w, dtype=np.float32)[0]  # [H]

    w1t = np.ascontiguousarray(w1.T).astype(io_np)  # [h, o]
    # cbias[b, o] = b1[o] + b2[o] + hidden[b] @ w2[o]
    cb = b1[None, :] + b2[None, :] + hid @ w2.T  # [B, O]
    maskneg_f = msk.astype(np.float32) * NEG  # [B, L]

    in_maps = []
    for c in range(NCORES):
        bs = slice(c * B_LOC, (c + 1) * B_LOC)
        cbc = cb[bs]  # [B_LOC, O]
        if LAYOUT == "a":
            mn = np.ascontiguousarray(
                maskneg_f[bs]
                .reshape(B_LOC, NSUP // B_LOC, SUBS, P)
                .transpose(2, 0, 1, 3)
                .reshape(SUBS, NSUP, P)
            )
            in_maps.append(
                {
                    "enc": np.ascontiguousarray(enc[bs].reshape(ROWS, H)).astype(
                        io_np
                    ),
                    "w1t": w1t,
                    "cbias": np.ascontiguousarray(cbc).astype(np.float32),
                    "vt": v.reshape(1, H).astype(io_np),
                    "maskneg": mn,
                }
            )
        else:
            # cbias tile layout [p, oc, b] with o = oc*128 + p
            cbias = np.ascontiguousarray(
                cbc.reshape(B_LOC, OC, P).transpose(2, 1, 0)
            ).astype(np.float32)
            vt = np.ascontiguousarray(v.reshape(OC, P).T).astype(io_np)  # [P, OC]
            in_maps.append(
                {
                    "enc": np.ascontiguousarray(enc[bs].reshape(ROWS, H)).astype(
                        io_np
                    ),
                    "w1t": w1t,
                    "cbias": cbias,
                    "vt": vt,
                    "maskneg": np.ascontiguousarray(maskneg_f[bs]).astype(io_np),
                }
            )
    return in_maps


_CACHE = {}


def _gather_core_out(arr: np.ndarray) -> np.ndarray:
    """Per-core device output -> [B_LOC, L] float32."""
    if LAYOUT == "a":
        # [lt, s, p] -> [b, l]
        return (
            arr.reshape(SUBS, B_LOC, NSUP // B_LOC, P)
            .transpose(1, 2, 0, 3)
            .reshape(B_LOC, L)
        )
    return arr.reshape(B_LOC, L)


def run(inputs: dict, trace: bool = False, tmpdir: str | None = None):
    from concourse.bass_utils import run_bass_kernel_spmd

    in_maps = _prep_in_maps(**inputs)
    if "nc" not in _CACHE:
        _CACHE["nc"] = _build()
    nc = _CACHE["nc"]
    res = run_bass_kernel_spmd(
        nc,
        in_maps,
        core_ids=list(range(NCORES)),
        trace=trace,
        tmpdir=tmpdir,
    )
    out = np.concatenate(
        [_gather_core_out(res.results[i]["out"]) for i in range(NCORES)], axis=0
    )
    return out.astype(np.float32), res.exec_time_ns


def kernel(**inputs) -> np.ndarray:
    return run(inputs, trace=False)[0]


def bench(inputs: dict, iters: int = 32):
    """Run the kernel on all 8 cores, verify once, then time `iters`
    pipelined executions with device-resident inputs. Returns
    (out, per_call_ns)."""
    import time

    import jax
    from jax.experimental.shard_map import shard_map
    from jax.sharding import Mesh, NamedSharding, PartitionSpec

    from concourse import bass2jax

    bass2jax.install_neuronx_cc_hook()

    in_maps = _prep_in_maps(**inputs)
    if "nc" not in _CACHE:
        t_b = time.perf_counter()
        _CACHE["nc"] = _build()
        print(f"[bench] build+schedule: {time.perf_counter() - t_b:.1f} s")
    nc = _CACHE["nc"]

    import concourse.mybir as mybir

    partition_name = nc.partition_id_tensor.name if nc.partition_id_tensor else None
    in_names, out_names, out_avals, zero_outs = [], [], [], []
    has_partition = False
    for alloc in nc.m.functions[0].allocations:
        if not isinstance(alloc, mybir.MemoryLocationSet):
            continue
        name = alloc.memorylocations[0].name
        if alloc.kind == "ExternalInput":
            if name == partition_name or name == "partition_id":
                has_partition = True
            else:
                in_names.append(name)
        elif alloc.kind == "ExternalOutput":
            out_names.append(name)
            shape = tuple(alloc.tensor_shape)
            dtype = mybir.dt.np(alloc.dtype)
            out_avals.append(jax.core.ShapedArray(shape, dtype))
            zero_outs.append(np.zeros(shape, dtype))
    n_params = len(in_names)
    n_outs = len(out_avals)
    all_in_names = list(in_names) + out_names
    if has_partition:
        all_in_names.append(partition_name or "partition_id")
    # No donation: this kernel writes every element of every output, so the
    # zero "output operands" can be reused across timing iterations.
    donate = ()

    def _body(*args):
        ops = list(args)
        if has_partition:
            ops.append(bass2jax.partition_id_tensor())
        outs = bass2jax._bass_exec_p.bind(
            *ops,
            out_avals=tuple(out_avals),
            in_names=tuple(all_in_names),
            out_names=tuple(out_names),
            lowering_input_output_aliases=(),
            sim_require_finite=True,
            sim_require_nnan=True,
            nc=nc,
        )
        return tuple(outs)

    devices = jax.devices()[:NCORES]
    mesh = Mesh(np.asarray(devices), ("core",))
    in_specs = (PartitionSpec("core"),) * (n_params + n_outs)
    out_specs = (PartitionSpec("core"),) * n_outs
    sharded = jax.jit(
        shard_map(
            _body, mesh=mesh, in_specs=in_specs, out_specs=out_specs, check_rep=False
        ),
        donate_argnums=donate,
        keep_unused=True,
    )
    sh = NamedSharding(mesh, PartitionSpec("core"))
    concat_in = [
        jax.device_put(
            np.concatenate([in_maps[c][k] for c in range(NCORES)], axis=0), sh
        )
        for k in in_names
    ]

    def fresh_zeros():
        return [
            jax.device_put(np.zeros((NCORES * z.shape[0], *z.shape[1:]), z.dtype), sh)
            for z in zero_outs
        ]

    # first call: compile + correctness output
    t_c0 = time.perf_counter()
    out_arrs = sharded(*concat_in, *fresh_zeros())
    out_raw = np.asarray(out_arrs[out_names.index("out")])
    per_core_shape = out_raw.shape
    out_np = out_raw.reshape(NCORES, per_core_shape[0] // NCORES, *per_core_shape[1:])
    out = np.concatenate(
        [_gather_core_out(out_np[c]) for c in range(NCORES)], axis=0
    ).astype(np.float32)
    if "ver" in out_names:
        ver = np.asarray(out_arrs[out_names.index("ver")]).ravel()
        print(f"[bench] ver marker on device: {ver[:8]}")
    print(f"[bench] first call (incl compile): {time.perf_counter() - t_c0:.1f} s")

    # warmup a couple more
    for _ in range(3):
        r = sharded(*concat_in, *fresh_zeros())
    jax.block_until_ready(r)

    # Time two loop lengths; the marginal slope removes the fixed
    # dispatch/tunnel overhead and leaves per-execution device time.
    # min-of-repeats suppresses tunnel latency noise; large delta-N makes
    # the residual fixed-cost variance negligible.
    zset = fresh_zeros()
    jax.block_until_ready(zset)

    def timed(n):
        t0 = time.perf_counter()
        rs = [sharded(*concat_in, *zset) for _ in range(n)]
        jax.block_until_ready(rs)
        return time.perf_counter() - t0

    n1, n2 = max(8, iters // 16), iters
    reps = 4
    t_n1 = min(timed(n1) for _ in range(reps))
    t_n2 = min(timed(n2) for _ in range(reps))
    per_call_ns = (t_n2 - t_n1) / (n2 - n1) * 1e9
    avg_ns = t_n2 / n2 * 1e9
    return out, per_call_ns, avg_ns

